# revision 54
# baseline (speedup 1.0000x reference)
"""CrossViewTokenBlock Trainium2 kernel (fp8 DoubleRow version).

Problem: two token streams [B=4, T=1024, D=768]; each stream gets
LN -> cross-attention (12 heads, rel-pos bias) -> residual -> LN -> MLP -> residual,
with queries from its own stream and K/V from the other stream.

Sharding: the two streams' outputs are independent given the two inputs,
so the 8 cores each take one (stream, batch) pair: cores 0-3 = stream 1 /
batch 0-3, cores 4-7 = stream 2 / batch 0-3.  No collectives.  A single
SPMD Bass program runs on all 8 cores; per-core data via the input maps.

Per-core design:
 - All large GEMMs run in fp8e4 (e4m3, max 240) with DoubleRow perf mode:
   operands laid out [P, k_subtiles, N], two 128-row k-subtiles contracted
   per instruction at 0.5 cycles/output-column (2x the bf16 rate).
   Power-of-2 scales keep descaling exact: weights x1024, LN outputs x16
   (folded into the rstd computation), V x32, attn-out x32.
 - Accuracy: W1/W2 ship as fp8 pairs (value + same-scale fp8 residual);
   both halves accumulate into the same PSUM group, halving the MLP's
   weight-quantization error at 2x fc1/fc2 matmul cost (still 1/2 of
   bf16).  Attention-path fp8 error is damped by the small attention
   contribution to the output.
 - logits stay regular matmuls (fp8 operands, K=64); the rel-pos bias
   reduces to a per-key bias -wr_h*j - c_h + 1.5 with c_h = max_j(-wr_h*j),
   keeping exp() in fp8 range; the shift cancels in the softmax divide.
 - The softmax denominator comes from a ones-column appended to V; the
   normalize step is one scalar_tensor_tensor: (av / den) + bias -> fp8.
 - LN rstd is computed batched (4 tiles -> one [P,4] op chain): phase 1
   uses one ACT Sqrt per batch; LN_f uses a DVE-only quake rsqrt (bitcast
   seed + 2 Newton steps) so no set-3 ACT table load lands in the exp or
   gelu streams (2-3 table loads total).  LN applies run on GPSIMD
   (SBUF->SBUF) and ACT; PSUM drains are DVE/ACT only (GPSIMD cannot
   address PSUM).
 - proj and fc2 produce token-major [q, D] PSUM tiles directly (N=768);
   residual adds are single fused DVE scalar_tensor_tensor ops and the
   output needs no final transpose.
 - PE transposes run in fp8 via an fp8 identity; each token tile's 6
   transposes pack into one PSUM bank and drain with a single strided
   copy (Pool or DVE, balanced per phase).
 - Issue order is the software pipeline (engine queues are in-order,
   though the tile scheduler may reorder independent work): tokens ship
   twice (f32 for the residual, bf16 copies that feed the LN stats early
   off a lighter DMA); per feature chunk the K/Q GEMMs feed their two
   heads' logits+exp immediately with AV trailing two heads behind;
   LN+QKV+attention share one 8-bank PSUM window; proj/LN_f/mT chains
   run in token halves aligned with the fc1 halves; W1 streams in during
   the attention tail to keep attention-phase SBUF slim.
"""

import numpy as np
import ml_dtypes

P = 128
T = 1024
D = 768
H = 12
HD = 64
HID = 3072
B = 4
EPS = 1e-6
NT = T // P      # 8 token tiles
ND = D // P      # 6 feature chunks
NH = HID // P    # 24 hidden chunks
NDP = ND // 2    # 3 feature chunk pairs (DoubleRow)
NHP = NH // 2    # 12 hidden chunk pairs
SCALE = HD ** -0.5

SW = 1024.0      # weight scale
SX = 16.0        # LN-output scale
SV = 32.0        # value scale
SHIFT = 1.5      # softmax exponent headroom shift
W1_SPLIT = False  # fc1 weight residual pass (accuracy vs ~15us PE)

BF = ml_dtypes.bfloat16
F8 = ml_dtypes.float8_e4m3


def build_nc():
    import concourse.mybir as mybir
    import concourse.tile as tile
    from concourse import bacc
    from concourse.masks import make_identity

    f32 = mybir.dt.float32
    bf16 = mybir.dt.bfloat16
    fp8 = mybir.dt.float8e4
    AF = mybir.ActivationFunctionType
    OP = mybir.AluOpType
    DR = mybir.MatmulPerfMode.DoubleRow

    nc = bacc.Bacc(None, target_bir_lowering=False)

    xs_d = nc.dram_tensor("xs", [T, D], f32, kind="ExternalInput")
    xsb_d = nc.dram_tensor("xsb", [T, D], bf16, kind="ExternalInput")
    xob_d = nc.dram_tensor("xob", [T, D], bf16, kind="ExternalInput")
    wq_d = nc.dram_tensor("wq", [D, D], fp8, kind="ExternalInput")
    wkv_d = nc.dram_tensor("wkv", [D, 2 * D], fp8, kind="ExternalInput")
    wp_d = nc.dram_tensor("wp", [D, D], fp8, kind="ExternalInput")
    w1a_d = nc.dram_tensor("w1a", [D, HID], fp8, kind="ExternalInput")
    w1r_d = nc.dram_tensor("w1r", [D, HID], fp8, kind="ExternalInput")
    w2a_d = nc.dram_tensor("w2a", [HID, D], fp8, kind="ExternalInput")
    w2r_d = nc.dram_tensor("w2r", [HID, D], fp8, kind="ExternalInput")
    qb_d = nc.dram_tensor("qb", [P, ND], f32, kind="ExternalInput")
    kb_d = nc.dram_tensor("kb", [P, ND], f32, kind="ExternalInput")
    bv_d = nc.dram_tensor("bv", [P, D], bf16, kind="ExternalInput")
    b1_d = nc.dram_tensor("b1", [P, NH], f32, kind="ExternalInput")
    b2r_d = nc.dram_tensor("b2r", [P, D], f32, kind="ExternalInput")
    relb_d = nc.dram_tensor("relb", [P, H * NT], f32, kind="ExternalInput")
    out_d = nc.dram_tensor("out", [T, D], f32, kind="ExternalOutput")

    with tile.TileContext(nc) as tc:
        with (
            tc.tile_pool(name="persist", bufs=1) as persist,
            tc.tile_pool(name="stats", bufs=8) as stats,
            tc.tile_pool(name="norm", bufs=4) as norm_p,
            tc.tile_pool(name="actT", bufs=2) as actT_p,
            tc.tile_pool(name="outp", bufs=3) as out_p,
            tc.tile_pool(name="mlpw", bufs=1) as mlpw,
        ):
            ident = persist.tile([P, P], fp8)
            make_identity(nc, ident)

            qb_sb = persist.tile([P, ND], f32)
            kb_sb = persist.tile([P, ND], f32)
            bv_sb = persist.tile([P, D], bf16)
            b1_sb = persist.tile([P, NH], f32)
            b2r_sb = persist.tile([P, D], f32)
            relb_sb = persist.tile([P, H * NT], f32)

            xs_sb = persist.tile([P, NT, D], f32)   # becomes tokens+attn in place
            xs_t = xs_d[:].rearrange("(t p) d -> p t d", p=P)

            def ln_stats4(srcs, st_dt=None, act_sqrt=True):
                """Stats for a group of 4 token tiles; ONE batched rstd so
                the ACT Sqrt count stays tiny (table-thrash avoidance).
                bf16 stats (2x DVE rate) are fine for the attention-damped
                phase-1 LNs. -> (mvg [P,4,2], rstdg [P,4], nmrg [P,4])."""
                mvg = stats.tile([P, 4, 2], f32, tag="mvg")
                for i, src in enumerate(srcs):
                    st = stats.tile([P, 2, 6], st_dt or f32, tag="st")
                    for s in range(2):
                        nc.vector.bn_stats(st[:, s, :],
                                           src[:, s * 384:(s + 1) * 384])
                    nc.vector.bn_aggr(mvg[:, i, :], st[:])
                vg = stats.tile([P, 4], f32, tag="vg")
                nc.vector.tensor_scalar(vg[:], mvg[:, :, 1], EPS, None,
                                        op0=OP.add)
                rstdg = stats.tile([P, 4], f32, tag="rstdg")
                if act_sqrt:
                    vr = stats.tile([P, 4], f32, tag="vr")
                    nc.vector.reciprocal(vr[:], vg[:])
                    # 16*rsqrt(var+eps) == sqrt(256/(var+eps))
                    nc.scalar.activation(rstdg[:], vr[:], AF.Sqrt, scale=256.0)
                else:
                    # DVE-only 16*rsqrt (quake seed + 2 Newton steps, the
                    # x16 folded into the magic exponent) -- keeps set-3
                    # ACT table loads out of the exp/gelu streams
                    i32 = mybir.dt.int32
                    ri = rstdg[:].bitcast(i32)
                    nc.vector.tensor_scalar(
                        ri, vg[:].bitcast(i32), 1, None,
                        op0=OP.arith_shift_right)
                    nc.vector.tensor_scalar(
                        ri, ri, -1, 0x613759df, op0=OP.mult, op1=OP.add)
                    tq = stats.tile([P, 4], f32, tag="tq")
                    for _ in range(2):
                        nc.vector.tensor_tensor(
                            tq[:], rstdg[:], rstdg[:], OP.mult)
                        nc.vector.tensor_tensor(tq[:], tq[:], vg[:], OP.mult)
                        nc.vector.tensor_scalar(
                            tq[:], tq[:], -1.0 / 512.0, 1.5,
                            op0=OP.mult, op1=OP.add)
                        nc.vector.tensor_tensor(
                            rstdg[:], rstdg[:], tq[:], OP.mult)
                nmrg = stats.tile([P, 4], f32, tag="nmrg")
                nc.vector.scalar_tensor_tensor(
                    nmrg[:], mvg[:, :, 0], -1.0, rstdg[:],
                    op0=OP.mult, op1=OP.mult)
                return mvg, rstdg, nmrg

            def ln_stats1(src):
                """Single-tile stats with the DVE-only rsqrt (phase C)."""
                mvg = stats.tile([P, 1, 2], f32, tag="mv1")
                st = stats.tile([P, 2, 6], f32, tag="st")
                for s in range(2):
                    nc.vector.bn_stats(st[:, s, :],
                                       src[:, s * 384:(s + 1) * 384])
                nc.vector.bn_aggr(mvg[:, 0, :], st[:])
                vg = stats.tile([P, 1], f32, tag="vg1")
                nc.vector.tensor_scalar(vg[:], mvg[:, :, 1], EPS, None,
                                        op0=OP.add)
                rstdg = stats.tile([P, 1], f32, tag="rstdg1")
                i32 = mybir.dt.int32
                ri = rstdg[:].bitcast(i32)
                nc.vector.tensor_scalar(
                    ri, vg[:].bitcast(i32), 1, -1,
                    op0=OP.arith_shift_right, op1=OP.mult)
                nc.vector.tensor_scalar(ri, ri, 0x613759df, None, op0=OP.add)
                tq = stats.tile([P, 1], f32, tag="tq1")
                for _ in range(2):
                    nc.vector.tensor_tensor(
                        tq[:], rstdg[:], rstdg[:], OP.mult)
                    nc.vector.tensor_tensor(tq[:], tq[:], vg[:], OP.mult)
                    nc.vector.tensor_scalar(
                        tq[:], tq[:], -1.0 / 512.0, 1.5,
                        op0=OP.mult, op1=OP.add)
                    nc.vector.tensor_tensor(
                        rstdg[:], rstdg[:], tq[:], OP.mult)
                nmrg = stats.tile([P, 1], f32, tag="nmrg1")
                nc.vector.scalar_tensor_tensor(
                    nmrg[:], mvg[:, :, 0], -1.0, rstdg[:],
                    op0=OP.mult, op1=OP.mult)
                return mvg, rstdg, nmrg

            def ln_apply(dst, src, mvg, rstdg, nmrg, i):
                # SBUF->SBUF; GPSIMD takes 3 of each 4-tile batch, ACT the
                # 4th (Identity(rstd*x - mean*rstd) — in every table set),
                # keeping the serial DVE stats queue free of applies
                if i == 3:
                    nc.scalar.activation(dst, src, AF.Identity,
                                         bias=nmrg[:, i:i + 1],
                                         scale=rstdg[:, i:i + 1])
                else:
                    nc.gpsimd.tensor_scalar(
                        dst, src, mvg[:, i, 0:1], rstdg[:, i:i + 1],
                        op0=OP.subtract, op1=OP.mult)

            def transpose_tile(dstT, src, t, pool, copy_eng):
                """dstT[:, :, tP:(t+1)P] <- transpose of token tile src [P, D]
                fp8. Six PE transposes pack one PSUM bank; one strided copy.
                (HW fp8 transposes write with element step 2, hence the
                trailing interleave dim.)"""
                ps = pool.tile([P, ND, P, 2], fp8, tag="tr")
                for c in range(ND):
                    nc.tensor.transpose(
                        ps[:, c, :, 0], src[:, c * P:(c + 1) * P],
                        ident[:])
                if copy_eng is nc.scalar:
                    nc.scalar.copy(
                        dstT[:, :, t * P:(t + 1) * P], ps[:, :, :, 0])
                else:
                    copy_eng.tensor_copy(
                        dstT[:, :, t * P:(t + 1) * P], ps[:, :, :, 0])

            with (
                tc.tile_pool(name="attw", bufs=1) as attw,
                tc.tile_pool(name="attn", bufs=1) as attn_p,
                tc.tile_pool(name="pTp", bufs=3) as pT_p,
                tc.tile_pool(name="psA_tr", bufs=1, space="PSUM") as psA_tr,
                tc.tile_pool(name="psA_mm", bufs=2, space="PSUM") as psA_mm,
                tc.tile_pool(name="psB_lg", bufs=2, space="PSUM") as psB_lg,
                tc.tile_pool(name="psB_av", bufs=1, space="PSUM") as psB_av,
                tc.tile_pool(name="xop", bufs=1) as xo_p,
            ):
                wq_sb = attw.tile([P, ND, D], fp8)
                wkv_sb = attw.tile([P, ND, 2 * D], fp8)
                wp_sb = mlpw.tile([P, ND, D], fp8)
                w2a_sb = mlpw.tile([P, NH, D], fp8)
                w2r_sb = mlpw.tile([P, NH, D], fp8)
                xob_sb = xo_p.tile([P, NT, D], bf16)
                xsb_sb = xo_p.tile([P, NT, D], bf16)
                xob_t = xob_d[:].rearrange("(t p) d -> p t d", p=P)
                xsb_t = xsb_d[:].rearrange("(t p) d -> p t d", p=P)

                # Two HWDGE queues: SP carries the kv-side tokens (first
                # LN batch); the ACT-triggered queue carries Wkv + q-side
                # tokens + Wq in parallel, so neither tokens nor weights
                # gate the first logits.
                for t in range(NT):
                    nc.sync.dma_start(xob_sb[:, t, :], xob_t[:, t, :])
                for t in range(NT):
                    nc.sync.dma_start(xsb_sb[:, t, :], xsb_t[:, t, :])
                nc.sync.dma_start(
                    wkv_sb[:], wkv_d[:].rearrange("(c p) n -> p c n", p=P))
                nc.sync.dma_start(
                    wq_sb[:], wq_d[:].rearrange("(c p) n -> p c n", p=P))
                nc.sync.dma_start(qb_sb[:], qb_d[:])
                nc.sync.dma_start(kb_sb[:], kb_d[:])
                nc.sync.dma_start(bv_sb[:], bv_d[:])
                nc.sync.dma_start(b1_sb[:], b1_d[:])
                nc.sync.dma_start(b2r_sb[:], b2r_d[:])
                nc.sync.dma_start(relb_sb[:], relb_d[:])
                nc.sync.dma_start(
                    wp_sb[:], wp_d[:].rearrange("(c p) n -> p c n", p=P))
                nc.sync.dma_start(
                    w2a_sb[:], w2a_d[:].rearrange("(c p) n -> p c n", p=P))
                nc.sync.dma_start(
                    w2r_sb[:], w2r_d[:].rearrange("(c p) n -> p c n", p=P))
                for t in range(NT):
                    nc.sync.dma_start(xs_sb[:, t, :], xs_t[:, t, :])

                qT = attn_p.tile([P, ND, T], fp8)
                kT = attn_p.tile([P, ND, T], fp8)
                vt = attn_p.tile([P, NT, H, HD + 1], fp8)
                aout_n = mlpw.tile([P, NT, D], fp8)
                nc.gpsimd.memset(vt[:, :, :, HD:HD + 1], 1.0)

                # ---- LN_q -> Q, LN_kv -> K (then V in the background) ----
                xqT = actT_p.tile([P, ND, T], fp8, tag="actT")
                xkvT = actT_p.tile([P, ND, T], fp8, tag="actT")
                for g in range(2):
                    ts4 = range(4 * g, 4 * g + 4)
                    mvg, rstdg, nmrg = ln_stats4(
                        [xob_sb[:, t, :] for t in ts4], bf16)
                    for i, t in enumerate(ts4):
                        xkv_n = norm_p.tile([P, D], fp8, tag="n")
                        ln_apply(xkv_n[:], xob_sb[:, t, :], mvg, rstdg,
                                 nmrg, i)
                        transpose_tile(xkvT, xkv_n, t, psA_tr, nc.scalar)
                for g in range(2):
                    ts4 = range(4 * g, 4 * g + 4)
                    mvg, rstdg, nmrg = ln_stats4(
                        [xsb_sb[:, t, :] for t in ts4], bf16)
                    for i, t in enumerate(ts4):
                        xq_n = norm_p.tile([P, D], fp8, tag="n")
                        ln_apply(xq_n[:], xsb_sb[:, t, :], mvg, rstdg,
                                 nmrg, i)
                        transpose_tile(xqT, xq_n, t, psA_tr, nc.scalar)

                def k_gemm(m):
                    for n2 in range(2):
                        ns = slice(n2 * 512, (n2 + 1) * 512)
                        ps = psA_mm.tile([P, 512], f32, tag="mm")
                        for cp in range(NDP):
                            nc.tensor.matmul(
                                ps[:],
                                wkv_sb[:, 2 * cp:2 * cp + 2, m * P:(m + 1) * P],
                                xkvT[:, 2 * cp:2 * cp + 2, ns],
                                start=(cp == 0), stop=(cp == NDP - 1),
                                perf_mode=DR,
                            )
                        nc.vector.tensor_scalar(
                            kT[:, m, ns], ps[:], 1.0 / (SW * SX),
                            kb_sb[:, m:m + 1], op0=OP.mult, op1=OP.add)

                def q_gemm(m):
                    for n2 in range(2):
                        ns = slice(n2 * 512, (n2 + 1) * 512)
                        ps = psA_mm.tile([P, 512], f32, tag="mm")
                        for cp in range(NDP):
                            nc.tensor.matmul(
                                ps[:],
                                wq_sb[:, 2 * cp:2 * cp + 2, m * P:(m + 1) * P],
                                xqT[:, 2 * cp:2 * cp + 2, ns],
                                start=(cp == 0), stop=(cp == NDP - 1),
                                perf_mode=DR,
                            )
                        nc.vector.tensor_scalar(
                            qT[:, m, ns], ps[:], 1.0 / (SW * SX),
                            qb_sb[:, m:m + 1], op0=OP.mult, op1=OP.add)

                def v_gemm(kb):
                    for off, nsz, h0, nh in ((0, 512, 0, 8), (512, 256, 8, 4)):
                        ps = psA_mm.tile([P, 512], f32, tag="mm")
                        for cp in range(NDP):
                            nc.tensor.matmul(
                                ps[:, :nsz],
                                xkvT[:, 2 * cp:2 * cp + 2, kb * P:(kb + 1) * P],
                                wkv_sb[:, 2 * cp:2 * cp + 2,
                                       D + off:D + off + nsz],
                                start=(cp == 0), stop=(cp == NDP - 1),
                                perf_mode=DR,
                            )
                        nc.vector.tensor_scalar(
                            vt[:, kb, h0:h0 + nh, 0:HD],
                            ps[:, :nsz].rearrange("p (h e) -> p h e", e=HD),
                            SV / (SW * SX), None, op0=OP.mult)

                def logits_exp(h):
                    hs = slice((h % 2) * HD, (h % 2) * HD + HD)
                    hc = h // 2
                    pT = pT_p.tile([P, NT, T], fp8, tag="pT")
                    for kt in range(NT):
                        lg = psB_lg.tile([P, T], f32, tag="lg")
                        for n2 in range(2):
                            ns = slice(n2 * 512, (n2 + 1) * 512)
                            nc.tensor.matmul(
                                lg[:, ns], kT[hs, hc, kt * P:(kt + 1) * P],
                                qT[hs, hc, ns], start=True, stop=True,
                            )
                        ih = h * NT + kt
                        nc.scalar.activation(
                            pT[:, kt, :], lg[:], AF.Exp,
                            bias=relb_sb[:, ih:ih + 1], scale=SCALE,
                        )
                    return pT

                def av_phase(h, pT):
                    for qb in range(NT):
                        av = psB_av.tile([P, HD + 1], f32, tag="av")
                        for kp in range(NT // 2):
                            nc.tensor.matmul(
                                av[:],
                                pT[:, 2 * kp:2 * kp + 2, qb * P:(qb + 1) * P],
                                vt[:, 2 * kp:2 * kp + 2, h, :],
                                start=(kp == 0), stop=(kp == NT // 2 - 1),
                                perf_mode=DR,
                            )
                        # aout = av/den + bv  (all x32), fp8 out
                        rs = stats.tile([P, 1], f32, tag="rs")
                        nc.vector.reciprocal(rs[:], av[:, HD:HD + 1])
                        nc.vector.scalar_tensor_tensor(
                            aout_n[:, qb, h * HD:(h + 1) * HD],
                            av[:, 0:HD], rs[:],
                            bv_sb[:, h * HD:(h + 1) * HD],
                            op0=OP.mult, op1=OP.add,
                        )

                # Issue order = software pipeline: each feature chunk's
                # K/Q GEMMs feed its two heads' logits+exp immediately; V
                # GEMMs slot in after chunk 1 (needed only by the first AV);
                # AV phases trail two heads behind so exps never starve.
                pTs = {}
                for m in range(ND):
                    k_gemm(m)
                    q_gemm(m)
                    pTs[2 * m] = logits_exp(2 * m)
                    pTs[2 * m + 1] = logits_exp(2 * m + 1)
                    if m == 1:
                        for kb in range(NT):
                            v_gemm(kb)
                        for h in (0, 1):
                            av_phase(h, pTs.pop(h))
                    if m >= 2:
                        for h in (2 * m - 2, 2 * m - 1):
                            av_phase(h, pTs.pop(h))
                for h in range(H - 2, H):
                    av_phase(h, pTs.pop(h))

            # ---- proj (token-major) + LN_f + mT, staggered; fc1+gelu ----
            with (
                tc.tile_pool(name="gTp", bufs=1) as gT_p,
                tc.tile_pool(name="mlpw1", bufs=1) as mlpw1,
                tc.tile_pool(name="psD_mm", bufs=3, space="PSUM") as psD_mm,
            ):
                # W1 lands during the attention tail; its SBUF would
                # otherwise crowd out the attention working set
                w1a_sb = mlpw1.tile([P, ND, HID], fp8)
                nc.sync.dma_start(
                    w1a_sb[:], w1a_d[:].rearrange("(c p) n -> p c n", p=P))
                if W1_SPLIT:
                    w1r_sb = mlpw1.tile([P, ND, HID], fp8)
                    nc.sync.dma_start(
                        w1r_sb[:],
                        w1r_d[:].rearrange("(c p) n -> p c n", p=P))
                w1_list = (w1a_sb, w1r_sb) if W1_SPLIT else (w1a_sb,)
                with (
                    tc.tile_pool(name="psC_tr", bufs=2, space="PSUM") as psC_tr,
                    tc.tile_pool(name="psC_mm", bufs=1, space="PSUM") as psC_mm,
                ):
                    aoutT = actT_p.tile([P, ND, T], fp8, tag="actT")
                    mT = actT_p.tile([P, ND, T], fp8, tag="actT")

                    def proj_chain(qb, copy_eng):
                        transpose_tile(aoutT, aout_n[:, qb, :], qb, psC_tr,
                                       copy_eng)
                        ps = psC_mm.tile([P, D], f32, tag="mm")
                        for off, nsz in ((0, 512), (512, 256)):
                            for cp in range(NDP):
                                nc.tensor.matmul(
                                    ps[:, off:off + nsz],
                                    aoutT[:, 2 * cp:2 * cp + 2,
                                          qb * P:(qb + 1) * P],
                                    wp_sb[:, 2 * cp:2 * cp + 2,
                                          off:off + nsz],
                                    start=(cp == 0), stop=(cp == NDP - 1),
                                    perf_mode=DR,
                                )
                        nc.vector.scalar_tensor_tensor(
                            xs_sb[:, qb, :], ps[:], 1.0 / (SV * SW),
                            xs_sb[:, qb, :], op0=OP.mult, op1=OP.add)

                    # token halves align with fc1 halves: LN_f batch g feeds
                    # fc1_half(g) as soon as its four mT tiles land
                    for g in range(2):
                        ts4 = range(4 * g, 4 * g + 4)
                        # batch 1 copies go to DVE so the ACT queue stays
                        # clear for the fc1-half-0 gelu stream
                        ce = nc.scalar if g == 0 else nc.vector
                        for qb in ts4:
                            proj_chain(qb, ce)
                        mvg, rstdg, nmrg = ln_stats4(
                            [xs_sb[:, t, :] for t in ts4], act_sqrt=False)
                        for i, t in enumerate(ts4):
                            m_n = norm_p.tile([P, D], fp8, tag="n")
                            ln_apply(m_n[:], xs_sb[:, t, :], mvg, rstdg,
                                     nmrg, i)
                            transpose_tile(mT, m_n, t, psC_tr, ce)

                # ---- fc1+gelu and fc2 (token-major) + residual + store ----
                with tc.tile_pool(name="psD_o", bufs=2, space="PSUM") as psD_o:
                    gT = gT_p.tile([P, NH, T], fp8)

                    def fc1_half(half):
                        ts_ = slice(half * 512, (half + 1) * 512)
                        for m in range(NH):
                            ps = psD_mm.tile([P, 512], f32, tag="mm")
                            for i, w1x in enumerate(w1_list):
                                for cp in range(NDP):
                                    nc.tensor.matmul(
                                        ps[:],
                                        w1x[:, 2 * cp:2 * cp + 2,
                                            m * P:(m + 1) * P],
                                        mT[:, 2 * cp:2 * cp + 2, ts_],
                                        start=(i == 0 and cp == 0),
                                        stop=(i == len(w1_list) - 1
                                              and cp == NDP - 1),
                                        perf_mode=DR,
                                    )
                            nc.scalar.activation(
                                gT[:, m, ts_], ps[:], AF.Gelu_apprx_tanh,
                                bias=b1_sb[:, m:m + 1], scale=1.0 / (SW * SX))

                    def fc2_qb(qb):
                        ps = psD_o.tile([P, D], f32, tag="o")
                        for off, nsz in ((0, 512), (512, 256)):
                            for i, w2x in enumerate((w2a_sb, w2r_sb)):
                                for cc in range(NHP):
                                    nc.tensor.matmul(
                                        ps[:, off:off + nsz],
                                        gT[:, 2 * cc:2 * cc + 2,
                                           qb * P:(qb + 1) * P],
                                        w2x[:, 2 * cc:2 * cc + 2,
                                            off:off + nsz],
                                        start=(i == 0 and cc == 0),
                                        stop=(i == 1 and cc == NHP - 1),
                                        perf_mode=DR,
                                    )
                        ob = out_p.tile([P, D], f32, tag="ob")
                        for off, nsz in ((0, 512), (512, 256)):
                            sl = slice(off, off + nsz)
                            nc.vector.scalar_tensor_tensor(
                                ob[:, sl], ps[:, sl], 1.0 / SW,
                                xs_sb[:, qb, sl], op0=OP.mult, op1=OP.add)
                            nc.vector.tensor_tensor(
                                ob[:, sl], ob[:, sl], b2r_sb[:, sl], OP.add)
                            nc.sync.dma_start(
                                out_d[qb * P:(qb + 1) * P, sl], ob[:, sl])

                    fc1_half(0)
                    fc1_half(1)
                    for qb in range(NT):
                        fc2_qb(qb)

    nc.finalize()
    return nc


def make_in_maps(inputs):
    """Host-side prep: fold LN gammas/betas into weights/biases, quantize
    weights to fp8 (value + residual pairs for W1/W2), build 8 per-core
    input maps. cores 0-3: stream1 batch 0-3; 4-7: stream2."""
    inp = {k: np.asarray(v) for k, v in inputs.items()}
    f32 = np.float32

    def to8(x):
        return np.ascontiguousarray(
            np.clip(np.asarray(x, f32), -224.0, 224.0).astype(F8))

    def stream_tensors(tag, snum, gq, bq, gkv, bkv, gf, bf_):
        Wq, Wkv, Wp, Wr = (inp["Wq" + tag], inp["Wkv" + tag],
                           inp["Wp" + tag], inp["Wr" + tag])
        W1, b1v, W2, b2v = (inp["Wm" + snum + "a"], inp["bm" + snum + "a"],
                            inp["Wm" + snum + "b"], inp["bm" + snum + "b"])
        gq, bq, gkv, bkv, gf, bf_ = (inp[g].astype(f32) for g in
                                     (gq, bq, gkv, bkv, gf, bf_))
        qb = (bq @ Wq).astype(f32)
        kvb = (bkv @ Wkv).astype(f32)
        b1 = (b1v + bf_ @ W1).astype(f32)
        wr = Wr[0].astype(f32)
        relb = np.empty((P, H * NT), f32)
        kk = np.arange(P, dtype=f32)
        for h in range(H):
            c_h = max(0.0, -wr[h] * (T - 1.0))
            for kt in range(NT):
                relb[:, h * NT + kt] = -wr[h] * (kt * P + kk) - c_h + SHIFT

        W1g = (gf[:, None] * W1).astype(f32) * SW
        w1a = to8(W1g)
        w1r = to8(W1g - w1a.astype(f32))
        W2s = W2.astype(f32) * SW
        w2a = to8(W2s)
        w2r = to8(W2s - w2a.astype(f32))
        return {
            "wq": to8(gq[:, None] * Wq * SW),
            "wkv": to8(gkv[:, None] * Wkv * SW),
            "wp": to8(Wp * SW),
            "w1a": w1a, "w1r": w1r, "w2a": w2a, "w2r": w2r,
            "qb": np.ascontiguousarray(qb.reshape(ND, P).T),
            "kb": np.ascontiguousarray(kvb[:D].reshape(ND, P).T),
            "bv": np.ascontiguousarray(
                np.broadcast_to((SV * kvb[D:]).astype(BF), (P, D))),
            "b1": np.ascontiguousarray(b1.reshape(NH, P).T),
            "b2r": np.ascontiguousarray(
                np.broadcast_to(b2v.astype(f32), (P, D))),
            "relb": relb,
        }

    s1 = stream_tensors("12", "1", "g_q1", "b_q1", "g_kv1", "b_kv1",
                        "g_f1", "b_f1")
    s2 = stream_tensors("21", "2", "g_q2", "b_q2", "g_kv2", "b_kv2",
                        "g_f2", "b_f2")
    t1 = np.ascontiguousarray(inp["tokens1"].astype(f32))
    t2 = np.ascontiguousarray(inp["tokens2"].astype(f32))
    t1b = np.ascontiguousarray(t1.astype(BF))
    t2b = np.ascontiguousarray(t2.astype(BF))

    in_maps = []
    for b in range(B):
        in_maps.append({"xs": t1[b], "xsb": t1b[b], "xob": t2b[b], **s1})
    for b in range(B):
        in_maps.append({"xs": t2[b], "xsb": t2b[b], "xob": t1b[b], **s2})
    return in_maps


_NC_CACHE = []


def kernel(**inputs):
    from concourse.bass_utils import run_bass_kernel_spmd

    if not _NC_CACHE:
        _NC_CACHE.append(build_nc())
    nc = _NC_CACHE[0]
    in_maps = make_in_maps(inputs)
    res = run_bass_kernel_spmd(nc, in_maps, core_ids=list(range(2 * B)))
    r = res.results
    tokens1 = np.stack([r[b]["out"] for b in range(B)]).astype(np.float32)
    tokens2 = np.stack([r[B + b]["out"] for b in range(B)]).astype(np.float32)
    return tokens1, tokens2
